# revision 1
# baseline (speedup 1.0000x reference)
"""AttentionPooling (segment softmax pooling) on 8 Trainium2 NeuronCores.

z[b] = sum_i softmax_within_segment(alpha)_i * x_i  for segment b, where
alpha = tanh(x @ W1.T) @ W2.T.

Strategy (data parallel over segments):
- batch is sorted, B = 1024 = 8 * 128, so core c owns segments
  [128c, 128(c+1)) — a contiguous row range of x. No cross-core segments,
  so the host just concatenates per-core results.
- alpha for this distribution lies in ~[-3, 3], so exp() without the
  per-segment max subtraction is numerically safe; softmax = e / seg_sum(e).
- Per 128-row tile on device:
    yT   = W1 @ x_tile.T          (PE, bf16, K=256 via 2 chunks)
    th   = tanh(yT)               (ACT, PSUM->SBUF bf16)
    a    = th.T @ W2              (PE -> (128 rows x 1) PSUM)
    e    = exp(a)                 (ACT -> e_buf in SBUF, dumped at end)
    E    = (iota == colidx%32) * e (DVE, built for 4 tiles per op via
                                   stride-0 broadcast APs; one-hot is only
                                   32 wide = segment index mod 32)
    gpool += E.T @ x_tile         (PE, (32 x 256) per-group PSUM; a 32-tile
                                   group spans <= ~10 segments so mod-32 is
                                   collision-free within a group)
  ...and once per 32-tile group:
    pool += scatter_g.T @ gpool   (PE, f32 0/1 scatter matmul - exact -
                                   into the persistent (128 segs x 256) pool)
- Host: denominator d_s = segment_sum(bf16(e)) from the e dump (exactly
  the same bf16 values the E matrix used), z = pool / d.

x is shipped twice in bf16 (row-major for pooling, transposed for the
matmul contraction over D) — 2 bytes * 2 orientations = same HBM traffic
as reading the f32 x once; the kernel is HBM-bandwidth bound.
"""

import numpy as np
import ml_dtypes

import concourse.bacc as bacc
import concourse.mybir as mybir
import concourse.tile as tile
from concourse.bass_utils import run_bass_kernel_spmd

bf16 = ml_dtypes.bfloat16
F32 = mybir.dt.float32
BF16 = mybir.dt.bfloat16
AF = mybir.ActivationFunctionType
ALU = mybir.AluOpType

NCORES = 8
D = 256
H = 128
SEGS_PER_CORE = 128
GT = 32          # max tiles per DMA group; a group spans <= ~10 segments
QUAD = 4         # tiles per mm1/psum_y batch (N' = 512)
EW = 32          # one-hot width: local segment index mod EW within a group

_kernel_cache = {}


def _group_plan(nt):
    """Uniform GT-tile DMA groups (irregular group sizes measurably hurt
    the DMA/PE pipeline on hardware)."""
    assert nt % GT == 0
    return [GT] * (nt // GT)


def _build_kernel(nt):
    """Build + compile the per-core SPMD kernel for nt 128-row tiles."""
    assert nt % 8 == 0 and GT % QUAD == 0
    nc = bacc.Bacc("TRN2", target_bir_lowering=False, debug=False)

    x_nat_d = nc.dram_tensor("x_nat", [128, nt, D], BF16, kind="ExternalInput").ap()
    xt_d = nc.dram_tensor("xT", [128, 2, nt * 128], BF16, kind="ExternalInput").ap()
    ci_d = nc.dram_tensor("colidx", [128, nt], BF16, kind="ExternalInput").ap()
    w1t_d = nc.dram_tensor("W1T", [128, 2, H], BF16, kind="ExternalInput").ap()
    w2_d = nc.dram_tensor("W2c", [H, 1], BF16, kind="ExternalInput").ap()
    iota_d = nc.dram_tensor("iota", [128, EW], BF16, kind="ExternalInput").ap()
    sizes = _group_plan(nt)
    ngroups = len(sizes)
    scat_d = nc.dram_tensor("scat", [EW, ngroups, SEGS_PER_CORE], F32,
                            kind="ExternalInput").ap()
    out_d = nc.dram_tensor("out", [SEGS_PER_CORE, D], F32, kind="ExternalOutput").ap()
    e_out_d = nc.dram_tensor("e_out", [128, nt], BF16, kind="ExternalOutput").ap()

    with tile.TileContext(nc) as tc:
        with (
            tc.tile_pool(name="const", bufs=1) as constp,
            tc.tile_pool(name="xn", bufs=3) as xnp,
            tc.tile_pool(name="xt", bufs=3) as xtp,
            tc.tile_pool(name="th", bufs=3) as thp,
            tc.tile_pool(name="ee", bufs=4) as eep,
            tc.tile_pool(name="out", bufs=1) as outp,
            tc.tile_pool(name="psum_y", bufs=2, space="PSUM") as psumy,
            tc.tile_pool(name="psum_al", bufs=1, space="PSUM") as psumal,
            tc.tile_pool(name="psum_gp", bufs=2, space="PSUM") as psumgp,
            tc.tile_pool(name="psum_acc", bufs=1, space="PSUM") as psumacc,
        ):
            w1t_sb = constp.tile([128, 2, H], BF16)
            nc.default_dma_engine.dma_start(w1t_sb[:], w1t_d[:])
            w2_sb = constp.tile([H, 1], BF16)
            nc.default_dma_engine.dma_start(w2_sb[:], w2_d[:])
            iota_sb = constp.tile([128, EW], BF16)
            nc.default_dma_engine.dma_start(iota_sb[:], iota_d[:])
            ci_sb = constp.tile([128, nt], BF16)
            nc.default_dma_engine.dma_start(ci_sb[:], ci_d[:])
            scat_sb = constp.tile([EW, ngroups, SEGS_PER_CORE], F32)
            nc.default_dma_engine.dma_start(scat_sb[:], scat_d[:])
            e_buf = constp.tile([128, nt], BF16)

            pool_ps = psumacc.tile([SEGS_PER_CORE, D], F32)

            gstart = 0
            for g, gsize in enumerate(sizes):
                xn = xnp.tile([128, gsize, D], BF16, tag="xn",
                              padded_shape=[128, GT, D])
                nc.default_dma_engine.dma_start(
                    xn[:], x_nat_d[:, gstart:gstart + gsize, :])
                xt = xtp.tile([128, 2, gsize * 128], BF16, tag="xt",
                              padded_shape=[128, 2, GT * 128])
                nc.default_dma_engine.dma_start(
                    xt[:], xt_d[:, :, gstart * 128:(gstart + gsize) * 128])

                gp_ps = psumgp.tile([EW, D], F32, tag="gp")

                # mm1 W1-chunk-outer per half-group (2 quads = 2 PSUM banks)
                nhalf_quads = 2
                for half in range((gsize // QUAD) // nhalf_quads):
                    y_ps = [psumy.tile([128, QUAD * 128], F32, name=f"y{q}",
                                       tag=f"y{q}")
                            for q in range(nhalf_quads)]
                    for chunk in range(2):
                        for q in range(nhalf_quads):
                            qq = half * nhalf_quads + q
                            nc.tensor.matmul(
                                y_ps[q][:], w1t_sb[:, chunk, :],
                                xt[:, chunk, qq * QUAD * 128:(qq + 1) * QUAD * 128],
                                start=(chunk == 0), stop=(chunk == 1))

                    for q in range(nhalf_quads):
                        qq = half * nhalf_quads + q
                        th = thp.tile([128, QUAD * 128], BF16, tag="th")
                        nc.scalar.activation(th[:], y_ps[q][:], AF.Tanh)

                        al_ps = psumal.tile([128, QUAD], F32, tag="al")
                        for j in range(QUAD):
                            nc.tensor.matmul(al_ps[:, j:j + 1],
                                             th[:, j * 128:(j + 1) * 128],
                                             w2_sb[:], start=True, stop=True)
                        t0 = gstart + qq * QUAD
                        nc.scalar.activation(e_buf[:, t0:t0 + QUAD], al_ps[:], AF.Exp)

                        # one-hot(e-weighted) E for the whole quad in 2 DVE ops
                        # via stride-0 broadcast APs
                        S4 = eep.tile([128, QUAD, EW], BF16, tag="S4")
                        nc.vector.tensor_tensor(
                            S4[:],
                            ci_sb[:, t0:t0 + QUAD].broadcast_to([128, QUAD, EW]),
                            iota_sb[:, None, :].broadcast_to([128, QUAD, EW]),
                            ALU.is_equal)
                        E4 = eep.tile([128, QUAD, EW], BF16, tag="E4")
                        nc.vector.tensor_mul(
                            E4[:], S4[:],
                            e_buf[:, t0:t0 + QUAD].broadcast_to([128, QUAD, EW]))
                        for j in range(QUAD):
                            tg = qq * QUAD + j  # tile index within group
                            nc.tensor.matmul(gp_ps[:], E4[:, j, :], xn[:, tg, :],
                                             start=(tg == 0), stop=(tg == gsize - 1))

                # scatter the group pool into the global per-segment pool
                # (f32 matmul with a 0/1 scatter matrix — exact)
                gp_sb = eep.tile([EW, D], F32, tag="gp_sb")
                nc.scalar.activation(gp_sb[:], gp_ps[:], AF.Copy)
                nc.tensor.matmul(pool_ps[:], scat_sb[:, g, :], gp_sb[:],
                                 start=(g == 0), stop=(g == ngroups - 1))
                gstart += gsize

            pool_sb = outp.tile([SEGS_PER_CORE, D], F32)
            nc.scalar.activation(pool_sb[:], pool_ps[:], AF.Copy)
            nc.default_dma_engine.dma_start(out_d[:], pool_sb[:])
            nc.default_dma_engine.dma_start(e_out_d[:], e_buf[:])

    nc.compile()
    return nc


def _prep_core(x, batch, r0, r1, seg0, nt):
    """Host-side shard prep for one core: rows [r0, r1) own segments
    [seg0, seg0+128). Returns the per-core input map."""
    rows = r1 - r0
    pad_rows = nt * 128

    xb = np.zeros((pad_rows, D), dtype=bf16)
    xb[:rows] = x[r0:r1].astype(bf16)
    # (128, nt, D): partition p holds row t*128 + p
    x_nat = np.ascontiguousarray(xb.reshape(nt, 128, D).transpose(1, 0, 2))

    xtb = np.zeros((2, H, pad_rows), dtype=bf16)
    xtb.reshape(D, pad_rows)[:, :rows] = xb[:rows].T
    xT = np.ascontiguousarray(xtb.transpose(1, 0, 2))  # (128, 2, pad_rows)

    seg_local = np.full(pad_rows, -1, dtype=np.int64)
    seg_local[:rows] = batch[r0:r1] - seg0
    ci = np.where(seg_local < 0, -1.0, seg_local % EW).astype(np.float32)
    colidx = np.ascontiguousarray(ci.reshape(nt, 128).T).astype(bf16)  # (128, nt)

    # scatter matrices: scat[k, g, s] = 1 iff group g's pool row k holds
    # local segment s (k = s mod EW). A group spans <= ~10 consecutive
    # segments, so within a group the mod-EW mapping is collision free.
    sizes = _group_plan(nt)
    scat = np.zeros((EW, len(sizes), SEGS_PER_CORE), dtype=np.float32)
    gstart = 0
    for g, gsize in enumerate(sizes):
        segs = np.unique(seg_local[gstart * 128:(gstart + gsize) * 128])
        segs = segs[segs >= 0]
        assert segs.size <= EW, f"group {g} spans {segs.size} segments > EW"
        scat[segs % EW, g, segs] = 1.0
        gstart += gsize

    return {"x_nat": x_nat, "xT": xT, "colidx": colidx, "scat": scat}


def _shared_inputs(W1, W2):
    w1t = np.ascontiguousarray(
        W1.T.astype(bf16).reshape(2, H, H).transpose(1, 0, 2))  # (128, 2, H)
    w2c = np.ascontiguousarray(W2.reshape(H, 1).astype(bf16))
    iota = np.broadcast_to(
        np.arange(EW, dtype=np.float32), (128, EW)).astype(bf16)
    return {"W1T": w1t, "W2c": w2c, "iota": iota}


def _seg_starts(x, batch):
    s = np.searchsorted(batch, np.arange(0, NCORES * SEGS_PER_CORE + 1, SEGS_PER_CORE))
    s[0], s[-1] = 0, x.shape[0]
    return s


def build_in_maps(x, batch, nt):
    s = _seg_starts(x, batch)
    return [_prep_core(x, batch, int(s[c]), int(s[c + 1]), c * SEGS_PER_CORE, nt)
            for c in range(NCORES)]


def pick_nt(x, batch):
    s = _seg_starts(x, batch)
    nt = int(max(-(-(int(s[c + 1] - s[c])) // 128) for c in range(NCORES)))
    return -(-nt // GT) * GT


def kernel(x, batch, W1, W2, B):
    x = np.asarray(x)
    batch = np.asarray(batch)
    W1 = np.asarray(W1)
    W2 = np.asarray(W2)
    B = int(B)
    assert B == NCORES * SEGS_PER_CORE

    nt = pick_nt(x, batch)
    if nt not in _kernel_cache:
        _kernel_cache[nt] = _build_kernel(nt)
    nc = _kernel_cache[nt]

    shared = _shared_inputs(W1, W2)
    in_maps = build_in_maps(x, batch, nt)
    for m in in_maps:
        m.update(shared)

    res = run_bass_kernel_spmd(nc, in_maps, core_ids=list(range(NCORES)))

    seg_starts = _seg_starts(x, batch)
    z = np.empty((B, D), dtype=np.float32)
    for c in range(NCORES):
        num = res.results[c]["out"]  # (128, D)
        # denominator from the e dump, rounded exactly like the E matrix
        e = res.results[c]["e_out"].T.reshape(-1)  # row t*128+p -> e
        r0, r1 = int(seg_starts[c]), int(seg_starts[c + 1])
        seg_local = (batch[r0:r1] - c * SEGS_PER_CORE).astype(np.int64)
        e_rows = e[:r1 - r0].astype(np.float64)
        den = np.bincount(seg_local, weights=e_rows, minlength=SEGS_PER_CORE)
        den = np.where(den == 0.0, 1.0, den).astype(np.float32)
        z[c * SEGS_PER_CORE:(c + 1) * SEGS_PER_CORE] = num / den[:, None]
    return z



# revision 2
# speedup vs baseline: 1.0373x; 1.0373x over previous
"""AttentionPooling (segment softmax pooling) on 8 Trainium2 NeuronCores.

z[b] = sum_i softmax_within_segment(alpha)_i * x_i  for segment b, where
alpha = tanh(x @ W1.T) @ W2.T.

Strategy (data parallel over segments):
- batch is sorted, B = 1024 = 8 * 128, so core c owns segments
  [128c, 128(c+1)) — a contiguous row range of x. No cross-core segments,
  so the host just concatenates per-core results.
- alpha for this distribution lies in ~[-3, 3], so exp() without the
  per-segment max subtraction is numerically safe; softmax = e / seg_sum(e).
- HBM traffic is the bottleneck, so x ships once in fp16 (row-major, the
  value path) and once in fp8e4 (transposed, feeds only the attention
  logits — fp8 noise there only perturbs softmax weights slightly;
  measured end-to-end rel err 1.7e-2 < 2e-2).
- mm1 runs as TWO DoubleRow fp8 matmuls accumulating into one PSUM:
  W1 ships as A = fp8(16*W1) plus the residual B = fp8(16*W1 - A), so W1
  itself contributes only ~bf16-level error; tanh applies scale=1/16.
  DoubleRow contracts K=256 in one pass at 0.5 cycles/row.
- Per 128-row tile on device:
    yT   = (A + B) @ x_tile.T     (PE, 2 fp8 DoubleRow matmuls per quad)
    th   = tanh(yT / 16)          (ACT, PSUM->SBUF fp16)
    a    = th.T @ W2              (PE -> (128 rows x 1) PSUM per chunk)
    e    = exp(a)                 (ACT, batched per half-group)
    E    = (iota == colidx%8) * e (DVE, built for 8 tiles per op via
                                   stride-0 broadcast APs; one-hot is only
                                   EW=8 wide = segment index mod 8)
    gpool += E.T @ x_tile         (PE, (8 x 256) per-group PSUM; a 16-tile
                                   group spans <= 6 segments so mod-8 is
                                   collision-free within a group)
  ...and once per 16-tile group:
    pool += scatter_g.T @ gpool   (PE, fp16 0/1 scatter matmul - exact -
                                   into the persistent (128 segs x 256) pool)
- Host: denominator d_s = segment_sum(fp16(e)) from the e dump (exactly
  the same fp16 values the E matrix used), z = pool / d.
"""

import numpy as np
import ml_dtypes

import concourse.bacc as bacc
import concourse.mybir as mybir
import concourse.tile as tile
from concourse.bass_utils import run_bass_kernel_spmd

f16 = np.float16
f8 = ml_dtypes.float8_e4m3
F32 = mybir.dt.float32
F16 = mybir.dt.float16
F8E4 = mybir.dt.float8e4
AF = mybir.ActivationFunctionType
ALU = mybir.AluOpType
DR = mybir.MatmulPerfMode.DoubleRow

NCORES = 8
D = 256
H = 128
SEGS_PER_CORE = 128
GT = 16          # tiles per DMA group; a 16-tile group spans <= ~6 segments
QUAD = 4         # tiles per mm1/psum_y batch (N' = 512)
EW = 8           # one-hot width: local segment index mod EW within a group
W1_SCALE = 16.0  # W1 pre-scale so fp8(16*W1) stays in normal range

_kernel_cache = {}


def _build_kernel(nt):
    """Build + compile the per-core SPMD kernel for nt 128-row tiles."""
    assert nt % GT == 0 and GT == 2 * QUAD * 2
    ngroups = nt // GT
    nc = bacc.Bacc("TRN2", target_bir_lowering=False, debug=False)

    x_nat_d = nc.dram_tensor("x_nat", [128, nt, D], F16, kind="ExternalInput").ap()
    xt_d = nc.dram_tensor("xT", [128, 2, nt * 128], F8E4, kind="ExternalInput").ap()
    ci_d = nc.dram_tensor("colidx", [128, nt], F16, kind="ExternalInput").ap()
    w1a_d = nc.dram_tensor("W1A", [128, 2, H], F8E4, kind="ExternalInput").ap()
    w1b_d = nc.dram_tensor("W1B", [128, 2, H], F8E4, kind="ExternalInput").ap()
    w2_d = nc.dram_tensor("W2c", [H, 1], F16, kind="ExternalInput").ap()
    iota_d = nc.dram_tensor("iota", [128, EW], F16, kind="ExternalInput").ap()
    scat_d = nc.dram_tensor("scat", [EW, ngroups, SEGS_PER_CORE], F16,
                            kind="ExternalInput").ap()
    out_d = nc.dram_tensor("out", [SEGS_PER_CORE, D], F32, kind="ExternalOutput").ap()
    e_out_d = nc.dram_tensor("e_out", [128, nt], F16, kind="ExternalOutput").ap()

    with tile.TileContext(nc) as tc:
        with (
            tc.tile_pool(name="const", bufs=1) as constp,
            tc.tile_pool(name="xn", bufs=3) as xnp,
            tc.tile_pool(name="xt", bufs=3) as xtp,
            tc.tile_pool(name="th", bufs=3) as thp,
            tc.tile_pool(name="ee", bufs=4) as eep,
            tc.tile_pool(name="out", bufs=1) as outp,
            tc.tile_pool(name="psum_y", bufs=2, space="PSUM") as psumy,
            tc.tile_pool(name="psum_al", bufs=2, space="PSUM") as psumal,
            tc.tile_pool(name="psum_gp", bufs=2, space="PSUM") as psumgp,
            tc.tile_pool(name="psum_acc", bufs=1, space="PSUM") as psumacc,
        ):
            w1a_sb = constp.tile([128, 2, H], F8E4)
            nc.default_dma_engine.dma_start(w1a_sb[:], w1a_d[:])
            w1b_sb = constp.tile([128, 2, H], F8E4)
            nc.default_dma_engine.dma_start(w1b_sb[:], w1b_d[:])
            w2_sb = constp.tile([H, 1], F16)
            nc.default_dma_engine.dma_start(w2_sb[:], w2_d[:])
            iota_sb = constp.tile([128, EW], F16)
            nc.default_dma_engine.dma_start(iota_sb[:], iota_d[:])
            ci_sb = constp.tile([128, nt], F16)
            nc.default_dma_engine.dma_start(ci_sb[:], ci_d[:])
            scat_sb = constp.tile([EW, ngroups, SEGS_PER_CORE], F16)
            nc.default_dma_engine.dma_start(scat_sb[:], scat_d[:])
            e_buf = constp.tile([128, nt], F16)

            pool_ps = psumacc.tile([SEGS_PER_CORE, D], F32)

            for g in range(ngroups):
                gstart = g * GT
                xn = xnp.tile([128, GT, D], F16, tag="xn")
                nc.default_dma_engine.dma_start(
                    xn[:], x_nat_d[:, gstart:gstart + GT, :])
                xt = xtp.tile([128, 2, GT * 128], F8E4, tag="xt")
                nc.default_dma_engine.dma_start(
                    xt[:], xt_d[:, :, gstart * 128:(gstart + GT) * 128])

                gp_ps = psumgp.tile([EW, D], F32, tag="gp")
                al_ps = psumal.tile([128, GT], F32, tag="al")

                for q in range(GT // QUAD):
                    y_ps = psumy.tile([128, QUAD * 128], F32, tag="y")
                    xt_q = xt[:, :, q * QUAD * 128:(q + 1) * QUAD * 128]
                    nc.tensor.matmul(y_ps[:], w1a_sb[:], xt_q,
                                     start=True, stop=False, perf_mode=DR)
                    nc.tensor.matmul(y_ps[:], w1b_sb[:], xt_q,
                                     start=False, stop=True, perf_mode=DR)

                    th = thp.tile([128, QUAD * 128], F16, tag="th")
                    nc.scalar.activation(th[:], y_ps[:], AF.Tanh,
                                         scale=1.0 / W1_SCALE)
                    for j in range(QUAD):
                        c = q * QUAD + j
                        nc.tensor.matmul(al_ps[:, c:c + 1],
                                         th[:, j * 128:(j + 1) * 128],
                                         w2_sb[:], start=True, stop=True)

                    if q % 2 == 1:
                        # exp + e-weighted one-hot for the half-group
                        # (2 quads = 8 tiles) in 1 ACT + 2 DVE ops
                        h0 = (q - 1) * QUAD          # tile in group
                        t0 = gstart + h0             # tile global
                        HT = 2 * QUAD
                        nc.scalar.activation(e_buf[:, t0:t0 + HT],
                                             al_ps[:, h0:h0 + HT], AF.Exp)
                        S8 = eep.tile([128, HT, EW], F16, tag="S8")
                        nc.vector.tensor_tensor(
                            S8[:],
                            ci_sb[:, t0:t0 + HT].broadcast_to([128, HT, EW]),
                            iota_sb[:, None, :].broadcast_to([128, HT, EW]),
                            ALU.is_equal)
                        E8 = eep.tile([128, HT, EW], F16, tag="E8")
                        nc.vector.tensor_mul(
                            E8[:], S8[:],
                            e_buf[:, t0:t0 + HT].broadcast_to([128, HT, EW]))
                        for j in range(HT):
                            tg = h0 + j
                            nc.tensor.matmul(gp_ps[:], E8[:, j, :],
                                             xn[:, tg, :],
                                             start=(tg == 0),
                                             stop=(tg == GT - 1))

                # scatter the group pool into the global per-segment pool
                # (fp16 matmul with a 0/1 scatter matrix — exact)
                gp_sb = eep.tile([EW, D], F16, tag="gp_sb")
                nc.scalar.activation(gp_sb[:], gp_ps[:], AF.Copy)
                nc.tensor.matmul(pool_ps[:], scat_sb[:, g, :], gp_sb[:],
                                 start=(g == 0), stop=(g == ngroups - 1))

            pool_sb = outp.tile([SEGS_PER_CORE, D], F32)
            nc.scalar.activation(pool_sb[:], pool_ps[:], AF.Copy)
            nc.default_dma_engine.dma_start(out_d[:], pool_sb[:])
            nc.default_dma_engine.dma_start(e_out_d[:], e_buf[:])

    nc.compile()
    return nc


def _prep_core(x, batch, r0, r1, seg0, nt):
    """Host-side shard prep for one core: rows [r0, r1) own segments
    [seg0, seg0+128). Returns the per-core input map."""
    rows = r1 - r0
    pad_rows = nt * 128

    xb = np.zeros((pad_rows, D), dtype=f16)
    xb[:rows] = x[r0:r1].astype(f16)
    # (128, nt, D): partition p holds row t*128 + p
    x_nat = np.ascontiguousarray(xb.reshape(nt, 128, D).transpose(1, 0, 2))

    x8 = np.zeros((pad_rows, D), dtype=f8)
    x8[:rows] = x[r0:r1].astype(f8)
    # (128, 2, pad_rows): partition d' holds feature c*128 + d'
    xT = np.ascontiguousarray(
        x8.T.reshape(2, 128, pad_rows).transpose(1, 0, 2))

    seg_local = np.full(pad_rows, -1, dtype=np.int64)
    seg_local[:rows] = batch[r0:r1] - seg0
    ci = np.where(seg_local < 0, -1.0, seg_local % EW).astype(np.float32)
    colidx = np.ascontiguousarray(ci.reshape(nt, 128).T).astype(f16)  # (128, nt)

    # scatter matrices: scat[k, g, s] = 1 iff group g's pool row k holds
    # local segment s (k = s mod EW). A 16-tile group spans <= ~6
    # consecutive segments, so within a group mod-EW is collision free.
    ngroups = nt // GT
    scat = np.zeros((EW, ngroups, SEGS_PER_CORE), dtype=f16)
    for g in range(ngroups):
        segs = np.unique(seg_local[g * GT * 128:(g + 1) * GT * 128])
        segs = segs[segs >= 0]
        assert segs.size <= EW, f"group {g} spans {segs.size} segments > EW"
        scat[segs % EW, g, segs] = 1.0

    return {"x_nat": x_nat, "xT": xT, "colidx": colidx, "scat": scat}


def _shared_inputs(W1, W2):
    W1s = (W1_SCALE * W1).astype(np.float32)
    A = W1s.astype(f8)
    Bm = (W1s - A.astype(np.float32)).astype(f8)

    def pack(w):  # (H, D) -> (128, 2, H) with [d', c, h] = w[h, c*128+d']
        return np.ascontiguousarray(
            w.T.reshape(2, 128, H).transpose(1, 0, 2))

    w2c = np.ascontiguousarray(W2.reshape(H, 1).astype(f16))
    iota = np.broadcast_to(
        np.arange(EW, dtype=np.float32), (128, EW)).astype(f16)
    return {"W1A": pack(A), "W1B": pack(Bm), "W2c": w2c, "iota": iota}


def _seg_starts(x, batch):
    s = np.searchsorted(batch, np.arange(0, NCORES * SEGS_PER_CORE + 1, SEGS_PER_CORE))
    s[0], s[-1] = 0, x.shape[0]
    return s


def build_in_maps(x, batch, nt):
    s = _seg_starts(x, batch)
    return [_prep_core(x, batch, int(s[c]), int(s[c + 1]), c * SEGS_PER_CORE, nt)
            for c in range(NCORES)]


def pick_nt(x, batch):
    s = _seg_starts(x, batch)
    nt = int(max(-(-(int(s[c + 1] - s[c])) // 128) for c in range(NCORES)))
    return -(-nt // GT) * GT


def kernel(x, batch, W1, W2, B):
    x = np.asarray(x)
    batch = np.asarray(batch)
    W1 = np.asarray(W1)
    W2 = np.asarray(W2)
    B = int(B)
    assert B == NCORES * SEGS_PER_CORE

    nt = pick_nt(x, batch)
    if nt not in _kernel_cache:
        _kernel_cache[nt] = _build_kernel(nt)
    nc = _kernel_cache[nt]

    shared = _shared_inputs(W1, W2)
    in_maps = build_in_maps(x, batch, nt)
    for m in in_maps:
        m.update(shared)

    res = run_bass_kernel_spmd(nc, in_maps, core_ids=list(range(NCORES)))

    seg_starts = _seg_starts(x, batch)
    z = np.empty((B, D), dtype=np.float32)
    for c in range(NCORES):
        num = res.results[c]["out"]  # (128, D)
        # denominator from the e dump, rounded exactly like the E matrix
        e = res.results[c]["e_out"].T.reshape(-1)  # row t*128+p -> e
        r0, r1 = int(seg_starts[c]), int(seg_starts[c + 1])
        seg_local = (batch[r0:r1] - c * SEGS_PER_CORE).astype(np.int64)
        e_rows = e[:r1 - r0].astype(np.float64)
        den = np.bincount(seg_local, weights=e_rows, minlength=SEGS_PER_CORE)
        den = np.where(den == 0.0, 1.0, den).astype(np.float32)
        z[c * SEGS_PER_CORE:(c + 1) * SEGS_PER_CORE] = num / den[:, None]
    return z
